# revision 1
# baseline (speedup 1.0000x reference)
"""LIF spike (vanilla) Trainium2 kernel.

Reference recurrence over leading time dim T (per element):
    u_t = TAU * u_{t-1} * (1 - o_{t-1}) + x_t
    o_t = (u_t - VTH > 0) ? 1.0 : 0.0

Decomposed into 3 elementwise ops per time step on carried state
c = u * (u <= VTH):
    S1: u = (c mult TAU) add x_t        (scalar_tensor_tensor, skipped at t=0)
    S2: o = relu(u - VTH) > 0           (ACT Relu, bf16 out; host decodes)
    S3: c = (u is_le VTH) mult u        (scalar_tensor_tensor, skipped at t=T-1)

All compares are exact fp32, so the spike train matches the fp32 jax
reference bit-for-bit. On device S2 runs as ACT Relu(u - VTH) with a
bf16 output (any positive fp32 difference survives the downcast as a
positive bf16), halving store traffic; the host maps >0 to 1.0f.

Sharding: pure data parallel over batch dim B=64 -> 8 cores x 8 batches.
Per core: 32MiB in (f32) + 16MiB out (bf16) HBM traffic.
"""

import numpy as np

T = 8
B = 64
C = 128
H = 32
W = 32
NCORES = 8
BS = B // NCORES            # batches per core
N = BS * C * H * W          # 1,048,576 elements per time step per core
P = 128                     # SBUF partitions
F = 4096                    # tile free-dim (tile = [128, 4096] f32 = 2MB)
NCHUNK = N // (P * F)       # spatial chunks per core
TAU = 0.5
VTH = 0.99999

OUT_DTYPE = "bfloat16"      # DRAM spike repr: relu(u-VTH) in bf16; host maps >0 -> 1.0
ACCUM_LOAD = False          # SWDGE accumulating loads (broken at runtime; keep off)


def _build(nt=T, nchunk=NCHUNK, fdim=F, xb=3, ob=3, ub=2, cb=1,
           out_dtype=OUT_DTYPE, accum_load=ACCUM_LOAD):
    import concourse.bacc as bacc
    import concourse.mybir as mybir
    import concourse.tile as tile

    f32 = mybir.dt.float32
    odt = getattr(mybir.dt, out_dtype)
    alu = mybir.AluOpType
    nc = bacc.Bacc("TRN2", target_bir_lowering=False)
    x = nc.dram_tensor("x", [nt, nchunk, P, fdim], f32, kind="ExternalInput")
    o = nc.dram_tensor("o", [nt, nchunk, P, fdim], odt, kind="ExternalOutput")
    s2_act = out_dtype == "bfloat16"
    with tile.TileContext(nc) as tc:
        with (
            tc.tile_pool(name="const", bufs=1) as constp,
            tc.tile_pool(name="xp", bufs=xb) as xp,
            tc.tile_pool(name="opool", bufs=ob) as opl,
            tc.tile_pool(name="up", bufs=ub) as up,
            tc.tile_pool(name="cp", bufs=cb) as cp,
        ):
            nvth = constp.tile([P, 1], f32)
            nc.vector.memset(nvth[:], -VTH)
            for i in range(nchunk):
                ct = None
                for t in range(nt):
                    if t == 0:
                        u = xp.tile([P, fdim], f32)
                        nc.sync.dma_start(u[:], x[t, i])
                    elif accum_load:
                        # u := tau*c, then DMA adds x_t in-flight (CCE add)
                        u = up.tile([P, fdim], f32)
                        nc.vector.tensor_scalar_mul(u[:], ct[:], TAU)
                        nc.gpsimd.dma_start(u[:], x[t, i], accum_op=alu.add)
                    else:
                        xt = xp.tile([P, fdim], f32)
                        nc.sync.dma_start(xt[:], x[t, i])
                        u = up.tile([P, fdim], f32)
                        nc.vector.scalar_tensor_tensor(
                            u[:], ct[:], TAU, xt[:], alu.mult, alu.add
                        )
                    ot = opl.tile([P, fdim], odt)
                    if s2_act:
                        # spike iff relu(u - VTH) > 0; exact in fp32, and any
                        # positive fp32 survives the bf16 downcast as positive
                        nc.scalar.activation(
                            ot[:], u[:], mybir.ActivationFunctionType.Relu,
                            bias=nvth[:], scale=1.0,
                        )
                    else:
                        nc.vector.tensor_scalar(ot[:], u[:], VTH, None, alu.is_gt)
                    nc.sync.dma_start(o[t, i], ot[:])
                    if t < nt - 1:
                        ct = cp.tile([P, fdim], f32)
                        nc.vector.scalar_tensor_tensor(
                            ct[:], u[:], VTH, u[:], alu.is_le, alu.mult
                        )
    nc.finalize()
    return nc


def kernel(x):
    x = np.ascontiguousarray(np.asarray(x, dtype=np.float32))
    assert x.shape == (T, B, C, H, W), x.shape
    from concourse.bass_utils import run_bass_kernel_spmd

    nc = _build()
    in_maps = []
    for i in range(NCORES):
        s = np.ascontiguousarray(x[:, i * BS : (i + 1) * BS])
        in_maps.append({"x": s.reshape(T, NCHUNK, P, F)})
    res = run_bass_kernel_spmd(nc, in_maps, core_ids=list(range(NCORES)))
    out = np.empty((T, B, C, H, W), np.float32)
    for i, r in enumerate(res.results):
        out[:, i * BS : (i + 1) * BS] = _decode(r["o"]).reshape(T, BS, C, H, W)
    return out


def _decode(o):
    """Device spike repr -> f32 spike train (bf16 relu(u-VTH): spike iff >0)."""
    o = np.asarray(o)
    if o.dtype == np.float32:
        return o
    return (o > 0).astype(np.float32)



# revision 5
# speedup vs baseline: 1.1080x; 1.1080x over previous
"""LIF spike (vanilla) Trainium2 kernel.

Reference recurrence over leading time dim T (per element):
    u_t = TAU * u_{t-1} * (1 - o_{t-1}) + x_t
    o_t = (u_t - VTH > 0) ? 1.0 : 0.0

Decomposed into 3 elementwise ops per time step on carried state
c = u * (u <= VTH):
    S1: u = (c mult TAU) add x_t        (scalar_tensor_tensor, skipped at t=0)
    S2: o = relu(u - VTH) > 0           (ACT Relu, bf16 out; host decodes)
    S3: c = (u is_le VTH) mult u        (scalar_tensor_tensor, skipped at t=T-1)

All compares are exact fp32, so the spike train matches the fp32 jax
reference bit-for-bit. On device S2 runs as ACT Relu(u - VTH) with a
bf16 output (any positive fp32 difference survives the downcast as a
positive bf16), halving store traffic; the host maps >0 to 1.0f.

Sharding: pure data parallel over batch dim B=64 -> 8 cores x 8 batches.
Per core: 32MiB in (f32) + 16MiB out (bf16) HBM traffic.
"""

import numpy as np

T = 8
B = 64
C = 128
H = 32
W = 32
NCORES = 8
BS = B // NCORES            # batches per core
N = BS * C * H * W          # 1,048,576 elements per time step per core
P = 128                     # SBUF partitions
F = 4096                    # tile free-dim (tile = [128, 4096] f32 = 2MB)
NCHUNK = N // (P * F)       # spatial chunks per core
TAU = 0.5
VTH = 0.99999

OUT_DTYPE = "float8e5"      # DRAM spike repr: relu(1024*(u-VTH)) in fp8e5m2; host maps >0 -> 1.0
OUT_SCALE = 1024.0          # power-of-2 scale keeps sign exact; min pos diff 2^-24 -> 2^-14 (e5m2 min normal)
ACCUM_LOAD = False          # SWDGE accumulating loads (broken at runtime; keep off)


def _build(nt=T, nchunk=NCHUNK, fdim=F, xb=3, ob=3, ub=2, cb=1,
           out_dtype=OUT_DTYPE, accum_load=ACCUM_LOAD):
    import concourse.bacc as bacc
    import concourse.mybir as mybir
    import concourse.tile as tile

    f32 = mybir.dt.float32
    odt = getattr(mybir.dt, out_dtype)
    alu = mybir.AluOpType
    nc = bacc.Bacc("TRN2", target_bir_lowering=False)
    x = nc.dram_tensor("x", [nt, nchunk, P, fdim], f32, kind="ExternalInput")
    o = nc.dram_tensor("o", [nt, nchunk, P, fdim], odt, kind="ExternalOutput")
    s2_act = out_dtype in ("bfloat16", "float8e5")
    s2_scale = OUT_SCALE if out_dtype == "float8e5" else 1.0
    with tile.TileContext(nc) as tc:
        with (
            tc.tile_pool(name="const", bufs=1) as constp,
            tc.tile_pool(name="xp", bufs=xb) as xp,
            tc.tile_pool(name="opool", bufs=ob) as opl,
            tc.tile_pool(name="up", bufs=ub) as up,
            tc.tile_pool(name="cp", bufs=cb) as cp,
        ):
            nvth = constp.tile([P, 1], f32)
            nc.vector.memset(nvth[:], float(np.float32(VTH) * np.float32(-s2_scale)))
            for i in range(nchunk):
                ct = None
                for t in range(nt):
                    if t == 0:
                        u = xp.tile([P, fdim], f32)
                        nc.sync.dma_start(u[:], x[t, i])
                    elif accum_load:
                        # u := tau*c, then DMA adds x_t in-flight (CCE add)
                        u = up.tile([P, fdim], f32)
                        nc.vector.tensor_scalar_mul(u[:], ct[:], TAU)
                        nc.gpsimd.dma_start(u[:], x[t, i], accum_op=alu.add)
                    else:
                        xt = xp.tile([P, fdim], f32)
                        nc.sync.dma_start(xt[:], x[t, i])
                        u = up.tile([P, fdim], f32)
                        nc.vector.scalar_tensor_tensor(
                            u[:], ct[:], TAU, xt[:], alu.mult, alu.add
                        )
                    ot = opl.tile([P, fdim], odt)
                    if s2_act:
                        # spike iff relu(u - VTH) > 0; exact in fp32, and any
                        # positive fp32 survives the bf16 downcast as positive
                        nc.scalar.activation(
                            ot[:], u[:], mybir.ActivationFunctionType.Relu,
                            bias=nvth[:], scale=s2_scale,
                        )
                    else:
                        nc.vector.tensor_scalar(ot[:], u[:], VTH, None, alu.is_gt)
                    nc.sync.dma_start(o[t, i], ot[:])
                    if t < nt - 1:
                        ct = cp.tile([P, fdim], f32)
                        nc.vector.scalar_tensor_tensor(
                            ct[:], u[:], VTH, u[:], alu.is_le, alu.mult
                        )
    nc.finalize()
    return nc


def kernel(x):
    x = np.ascontiguousarray(np.asarray(x, dtype=np.float32))
    assert x.shape == (T, B, C, H, W), x.shape
    from concourse.bass_utils import run_bass_kernel_spmd

    nc = _build()
    in_maps = []
    for i in range(NCORES):
        s = np.ascontiguousarray(x[:, i * BS : (i + 1) * BS])
        in_maps.append({"x": s.reshape(T, NCHUNK, P, F)})
    res = run_bass_kernel_spmd(nc, in_maps, core_ids=list(range(NCORES)))
    out = np.empty((T, B, C, H, W), np.float32)
    for i, r in enumerate(res.results):
        out[:, i * BS : (i + 1) * BS] = _decode(r["o"]).reshape(T, BS, C, H, W)
    return out


def _decode(o):
    """Device spike repr -> f32 spike train (bf16 relu(u-VTH): spike iff >0)."""
    o = np.asarray(o)
    if o.dtype == np.float32:
        return o
    return (o > 0).astype(np.float32)



# revision 8
# speedup vs baseline: 1.1291x; 1.0190x over previous
"""LIF spike (vanilla) Trainium2 kernel — time-bit-packed output.

Reference recurrence over leading time dim T (per element):
    u_t = TAU * u_{t-1} * (1 - o_{t-1}) + x_t
    o_t = (u_t - VTH > 0) ? 1.0 : 0.0

Device-side structure per (chunk, t):
    S1: u_t = TAU * select(u_{t-1} <= VTH, u_{t-1}, 0) + x_t
        -- ONE custom DVE op (LIF_GATED_DECAY_ADD_ANT), registered below.
           select(u < nextafter(VTH), u, 0) == u * (u <= VTH) exactly, and
           TAU = 0.5 is a power of two so TAU*u is exact: u_t matches the
           fp32 jax reference bit-for-bit.
    S2: s_t = Sign(u_t - VTH) in {-1, 0, +1}  (ACT engine, bf16 out)
    S3: packed = 2*packed + s_t               (DVE STT in bf16; t=0 writes
        packed directly from S2)

After t=T-1, packed = sum_t s_t * 2^(T-1-t) in [-255, 255] (exact in bf16:
integers up to 2^8 are representable). One bf16 store per chunk replaces
T stores: output traffic drops 16 MiB -> 2 MiB per core. Host decode:
bits of (packed + 255) / 2, bit (T-1-t) = spike_t.

Sharding: pure data parallel over batch dim B=64 -> 8 cores x 8 batches.
Per core: 32 MiB in (f32) + 2 MiB out (bf16) HBM traffic.
"""

import numpy as np

T = 8
B = 64
C = 128
H = 32
W = 32
NCORES = 8
BS = B // NCORES            # batches per core
N = BS * C * H * W          # 1,048,576 elements per time step per core
P = 128                     # SBUF partitions
F = 4096                    # tile free-dim (tile = [128, 4096] f32 = 2MB)
NCHUNK = N // (P * F)       # spatial chunks per core
TAU = 0.5
VTH = 0.99999
VTH_PLUS = float(np.nextafter(np.float32(VTH), np.float32(np.inf)))

HORNER_ENGINE = "vector"    # which engine runs the bf16 Horner STT


def _register_lif_op():
    """Register the fused LIF decay custom DVE op (idempotent).

    out = select(in0 < s0, in0, 0) * s1 + in1
    """
    from concourse import dve_ops
    from concourse.dve_spec import C0, C1, Spec, Src0, Src1, Zero, select
    from concourse.dve_spec import _has_src1, lower
    from concourse.dve_uop import DveOpSpec

    name = "LIF_GATED_DECAY_ADD_ANT"
    for op in dve_ops.OPS:
        if op.name == name:
            return op
    spec = Spec(
        body=select(Src0 < C0, Src0, Zero) * C1 + Src1,
        reference=lambda in0, in1, s0, s1, imm2: (
            np.where(in0 < s0, in0, np.float32(0.0)).astype(np.float32)
            * np.float32(s1)
            + in1
        ).astype(np.float32),
    )
    row = dve_ops._CUSTOM_DVE_ROW_BASE + len(dve_ops.OPS)
    assert row < 0x20, "custom-DVE opcode rows exhausted"
    shas = {}
    for ver in ("v3", "v4"):
        tmp = DveOpSpec(
            name=name, opcode=row, uops=lower(spec, ver=ver),
            rd1_en=_has_src1(spec),
        )
        shas[ver] = tmp.sha(ver)
    op = dve_ops.DveOp(name, spec, subdim=False, uops_sha=shas)
    dve_ops.OPS.append(op)
    dve_ops.CUSTOM_DVE_SPECS[name] = spec
    dve_ops._SUB_OPCODE_FOR_NAME[name] = row
    return op


def _build(nt=T, nchunk=NCHUNK, fdim=F, xb=4, ub=2, sb=3, pb=2):
    import concourse.bacc as bacc
    import concourse.mybir as mybir
    import concourse.tile as tile

    lif_op = _register_lif_op()

    f32 = mybir.dt.float32
    bf16 = mybir.dt.bfloat16
    alu = mybir.AluOpType
    act = mybir.ActivationFunctionType
    nc = bacc.Bacc("TRN2", target_bir_lowering=False)
    x = nc.dram_tensor("x", [nt, nchunk, P, fdim], f32, kind="ExternalInput")
    o = nc.dram_tensor("o", [nchunk, P, fdim], bf16, kind="ExternalOutput")
    horner = getattr(nc, HORNER_ENGINE)
    with tile.TileContext(nc) as tc:
        with (
            tc.tile_pool(name="const", bufs=1) as constp,
            tc.tile_pool(name="xp", bufs=xb) as xp,
            tc.tile_pool(name="up", bufs=ub) as up,
            tc.tile_pool(name="sp", bufs=sb) as sp,
            tc.tile_pool(name="pp", bufs=pb) as pp,
        ):
            nvth = constp.tile([P, 1], f32)
            nc.vector.memset(nvth[:], -VTH)
            for i in range(nchunk):
                packed = pp.tile([P, fdim], bf16)
                u = None
                st_prev = None
                for t in range(nt):
                    xt = xp.tile([P, fdim], f32)
                    nc.sync.dma_start(xt[:], x[t, i])
                    if t == 0:
                        u = xt
                    else:
                        un = up.tile([P, fdim], f32)
                        nc.vector._custom_dve(
                            lif_op, out=un[:], in0=u[:], in1=xt[:],
                            s0=VTH_PLUS, s1=TAU,
                        )
                        u = un
                        # Horner for t-1 issues AFTER the t state update so
                        # the in-order DVE queue never waits on ACT's sign.
                        if st_prev is not None:
                            horner.scalar_tensor_tensor(
                                packed[:], packed[:], 2.0, st_prev[:],
                                alu.mult, alu.add,
                            )
                    if t == 0:
                        # packed = sign(u_0 - VTH) directly
                        nc.scalar.activation(
                            packed[:], u[:], act.Sign, bias=nvth[:], scale=1.0,
                        )
                        st_prev = None
                    else:
                        st = sp.tile([P, fdim], bf16)
                        nc.scalar.activation(
                            st[:], u[:], act.Sign, bias=nvth[:], scale=1.0,
                        )
                        st_prev = st
                # final horner for t = nt-1
                horner.scalar_tensor_tensor(
                    packed[:], packed[:], 2.0, st_prev[:], alu.mult, alu.add,
                )
                nc.sync.dma_start(o[i], packed[:])
    nc.finalize()
    return nc


def kernel(x):
    x = np.ascontiguousarray(np.asarray(x, dtype=np.float32))
    assert x.shape == (T, B, C, H, W), x.shape
    from concourse.bass_utils import run_bass_kernel_spmd

    nc = _build()
    in_maps = []
    for i in range(NCORES):
        s = np.ascontiguousarray(x[:, i * BS : (i + 1) * BS])
        in_maps.append({"x": s.reshape(T, NCHUNK, P, F)})
    res = run_bass_kernel_spmd(nc, in_maps, core_ids=list(range(NCORES)))
    out = np.empty((T, B, C, H, W), np.float32)
    for i, r in enumerate(res.results):
        out[:, i * BS : (i + 1) * BS] = _decode(r["o"])
    return out


def _decode(o):
    """Packed bf16 sign-digit repr -> f32 spike train [T, BS, C, H, W].

    packed = sum_t s_t * 2^(T-1-t), s_t in {-1,+1} (0 only on exact
    threshold ties, measure-zero): bit (T-1-t) of (packed+255)/2 = spike_t.
    """
    v = np.asarray(o).astype(np.float32).reshape(-1)       # [NCHUNK*P*F]
    s = ((v + 255.0) * 0.5).astype(np.uint8)
    bits = np.unpackbits(s[:, None], axis=1, bitorder="big")  # [N, T]
    return (
        bits.T.astype(np.float32).reshape(T, BS, C, H, W)
    )


# revision 13
# speedup vs baseline: 1.3662x; 1.2100x over previous
"""LIF spike (vanilla) Trainium2 kernel — time-bit-packed output, PE pack.

Reference recurrence over leading time dim T (per element):
    u_t = TAU * u_{t-1} * (1 - o_{t-1}) + x_t
    o_t = (u_t - VTH > 0) ? 1.0 : 0.0

Device-side structure per (chunk, t):
    S1 (DVE):  u_t = TAU * select(u_{t-1} <= VTH, u_{t-1}, 0) + x_t
        -- ONE custom DVE op (LIF_GATED_DECAY_ADD_ANT), registered below.
           select(u < nextafter(VTH), u, 0) == u * (u <= VTH) exactly, and
           TAU = 0.5 is a power of two so TAU*u is exact: u_t matches the
           fp32 jax reference bit-for-bit.
    S2 (ACT):  s_t = Sign(u_t - VTH) in {-1, 0, +1}, bf16
    S3 (PE):   p += diag(2^(T-1-t)) @ s_t, accumulated in PSUM (fp32, exact)

After t=T-1, p = sum_t s_t * 2^(T-1-t) in [-255, 255]; one ACT copy
downcasts it to bf16 (integers up to 2^8 are exact in bf16) and one DMA
per chunk stores it: output traffic drops 16 MiB -> 2 MiB per core.
Host decode: bits of (p + 255) / 2, bit (T-1-t) = spike_t.

Engine budget per core (measured rates): DMA ~107 us (bound, ~334 GB/s),
DVE ~65 us, ACT ~63 us, PE ~32 us.

Sharding: pure data parallel over batch dim B=64 -> 8 cores x 8 batches.
Per core: 32 MiB in (f32) + 2 MiB out (bf16) HBM traffic.
"""

import numpy as np

T = 8
B = 64
C = 128
H = 32
W = 32
NCORES = 8
BS = B // NCORES            # batches per core
N = BS * C * H * W          # 1,048,576 elements per time step per core
P = 128                     # SBUF partitions
F = 2048                    # tile free-dim (tile = [128, 2048] f32 = 1MB)
NCHUNK = N // (P * F)       # spatial chunks per core
TAU = 0.5
VTH = 0.99999
VTH_PLUS = float(np.nextafter(np.float32(VTH), np.float32(np.inf)))
PSUM_BANK_F = 512           # f32 elements per partition per PSUM bank


def _digit_weights():
    """[T, 128, 128] bf16: W_t = diag(2^(T-1-t)) — PE pack weights."""
    import ml_dtypes

    w = np.zeros((T, P, P), np.float32)
    for t in range(T):
        w[t] = np.eye(P, dtype=np.float32) * float(2 ** (T - 1 - t))
    return w.astype(ml_dtypes.bfloat16)


def _register_lif_op():
    """Register the fused LIF decay custom DVE op (idempotent).

    out = select(in0 < s0, in0, 0) * s1 + in1
    """
    from concourse import dve_ops
    from concourse.dve_spec import C0, C1, Spec, Src0, Src1, Zero, select
    from concourse.dve_spec import _has_src1, lower
    from concourse.dve_uop import DveOpSpec

    name = "LIF_GATED_DECAY_ADD_ANT"
    for op in dve_ops.OPS:
        if op.name == name:
            return op
    spec = Spec(
        body=select(Src0 < C0, Src0, Zero) * C1 + Src1,
        reference=lambda in0, in1, s0, s1, imm2: (
            np.where(in0 < s0, in0, np.float32(0.0)).astype(np.float32)
            * np.float32(s1)
            + in1
        ).astype(np.float32),
    )
    row = dve_ops._CUSTOM_DVE_ROW_BASE + len(dve_ops.OPS)
    assert row < 0x20, "custom-DVE opcode rows exhausted"
    shas = {}
    for ver in ("v3", "v4"):
        tmp = DveOpSpec(
            name=name, opcode=row, uops=lower(spec, ver=ver),
            rd1_en=_has_src1(spec),
        )
        shas[ver] = tmp.sha(ver)
    op = dve_ops.DveOp(name, spec, subdim=False, uops_sha=shas)
    dve_ops.OPS.append(op)
    dve_ops.CUSTOM_DVE_SPECS[name] = spec
    dve_ops._SUB_OPCODE_FOR_NAME[name] = row
    return op


def _build(nt=T, nchunk=NCHUNK, fdim=F, xb=5, ub=2, sb=4, ob=2):
    import concourse.bacc as bacc
    import concourse.bass as bass
    import concourse.mybir as mybir
    import concourse.tile as tile

    lif_op = _register_lif_op()

    f32 = mybir.dt.float32
    bf16 = mybir.dt.bfloat16
    act = mybir.ActivationFunctionType
    nc = bacc.Bacc("TRN2", target_bir_lowering=False)
    x = nc.dram_tensor("x", [nt, nchunk, P, fdim], f32, kind="ExternalInput")
    w = nc.dram_tensor("w", [nt, P, P], bf16, kind="ExternalInput")
    o = nc.dram_tensor("o", [nchunk, P, fdim], bf16, kind="ExternalOutput")
    with tile.TileContext(nc) as tc:
        with (
            tc.tile_pool(name="const", bufs=1) as constp,
            tc.tile_pool(name="xp", bufs=xb) as xp,
            tc.tile_pool(name="up", bufs=ub) as up,
            tc.tile_pool(name="sp", bufs=sb) as sp,
            tc.tile_pool(name="op", bufs=ob) as op_,
            tc.tile_pool(name="pp", bufs=2, space=bass.MemorySpace.PSUM) as pp,
        ):
            nvth = constp.tile([P, 1], f32)
            nc.vector.memset(nvth[:], -VTH)
            wsb = constp.tile([P, nt, P], bf16)
            for t in range(nt):
                nc.sync.dma_start(wsb[:, t, :], w[t])
            for i in range(nchunk):
                p = pp.tile([P, fdim], f32)
                u = None
                for t in range(nt):
                    xt = xp.tile([P, fdim], f32)
                    nc.sync.dma_start(xt[:], x[t, i])
                    if t == 0:
                        u = xt
                    else:
                        un = up.tile([P, fdim], f32)
                        nc.vector._custom_dve(
                            lif_op, out=un[:], in0=u[:], in1=xt[:],
                            s0=VTH_PLUS, s1=TAU,
                        )
                        u = un
                    st = sp.tile([P, fdim], bf16)
                    nc.scalar.activation(
                        st[:], u[:], act.Sign, bias=nvth[:], scale=1.0,
                    )
                    # one Matmult may only target a single PSUM bank
                    # (512 f32 per partition): split across banks.
                    for j in range(fdim // PSUM_BANK_F):
                        sl = slice(j * PSUM_BANK_F, (j + 1) * PSUM_BANK_F)
                        nc.tensor.matmul(
                            p[:, sl], wsb[:, t, :], st[:, sl],
                            start=(t == 0), stop=(t == nt - 1),
                        )
                ot = op_.tile([P, fdim], bf16)
                nc.scalar.activation(ot[:], p[:], act.Copy)
                nc.sync.dma_start(o[i], ot[:])
    nc.finalize()
    return nc


def _in_maps(x):
    wdig = _digit_weights()
    in_maps = []
    for i in range(NCORES):
        s = np.ascontiguousarray(x[:, i * BS : (i + 1) * BS])
        in_maps.append({"x": s.reshape(T, NCHUNK, P, F), "w": wdig})
    return in_maps


def kernel(x):
    x = np.ascontiguousarray(np.asarray(x, dtype=np.float32))
    assert x.shape == (T, B, C, H, W), x.shape
    from concourse.bass_utils import run_bass_kernel_spmd

    nc = _build()
    res = run_bass_kernel_spmd(nc, _in_maps(x), core_ids=list(range(NCORES)))
    out = np.empty((T, B, C, H, W), np.float32)
    for i, r in enumerate(res.results):
        out[:, i * BS : (i + 1) * BS] = _decode(r["o"])
    return out


def _decode(o):
    """Packed bf16 sign-digit repr -> f32 spike train [T, BS, C, H, W].

    packed = sum_t s_t * 2^(T-1-t), s_t in {-1,+1} (0 only on exact
    threshold ties, measure-zero): bit (T-1-t) of (packed+255)/2 = spike_t.
    """
    v = np.asarray(o).astype(np.float32).reshape(-1)       # [NCHUNK*P*F]
    s = ((v + 255.0) * 0.5).astype(np.uint8)
    bits = np.unpackbits(s[:, None], axis=1, bitorder="big")  # [N, T]
    return bits.T.astype(np.float32).reshape(T, BS, C, H, W)
